# revision 12
# baseline (speedup 1.0000x reference)
"""CRFVGG_prune message-passing kernel for 8 TRN2 NeuronCores.

Structure: 3 node types with channel counts [228, 111, 51] hold [B,C,256,256]
feature maps. Two message-passing iterations of 1x1 convs (per-pixel matmuls)
between all ordered node pairs, each followed by prelu + residual + relu.

Sharding: pure data parallel over B*H rows (512 rows -> 64 rows/core).
Per-core layout is channels-on-partitions: x [512(pad), 16384px] bf16, where
channels are padded per-chunk to 128 (h0a|h0b|h1|h2) so every input DMA uses
all 128 partitions. All matmuls are bf16 (fp32 PSUM accum), elementwise in
bf16, output written bf16 and upcast on host.
"""
import os
import sys

sys.path.insert(0, "/opt/trn_rl_repo")

import numpy as np
import ml_dtypes

import concourse.bass as bass
import concourse.tile as tile
from concourse import bacc, mybir
from concourse.bass_utils import run_bass_kernel_spmd

BF16 = ml_dtypes.bfloat16
BF = mybir.dt.bfloat16
F32 = mybir.dt.float32

B, H, W = 2, 256, 256
CHS = [228, 111, 51]
CTOT = sum(CHS)  # 390
NCORES = 8
ROWS_PER_CORE = (B * H) // NCORES  # 64
PX = ROWS_PER_CORE * W  # 16384 pixels per core
MACRO = 2048  # pixels per macro tile
NMACRO = PX // MACRO
NSUB = MACRO // 512  # matmul N-subtiles per macro

# chunk: (name, out_row_start in 390-layout, n_rows, padded_row_start in 512-layout)
CHUNKS = {
    "0a": (0, 128, 0),
    "0b": (128, 100, 128),
    "1": (228, 111, 256),
    "2": (339, 51, 384),
}
# target chunk -> list of (source chunk, weight key, K-slice, M-slice)
# weight w_j_i: [cout_i, cin_j]; lhsT piece = w_j_i.T[kslice, mslice]
TARGETS = {
    "0a": [("1", "w_1_0", (0, 111), (0, 128)), ("2", "w_2_0", (0, 51), (0, 128))],
    "0b": [("1", "w_1_0", (0, 111), (128, 228)), ("2", "w_2_0", (0, 51), (128, 228))],
    "1": [
        ("0a", "w_0_1", (0, 128), (0, 111)),
        ("0b", "w_0_1", (128, 228), (0, 111)),
        ("2", "w_2_1", (0, 51), (0, 111)),
    ],
    "2": [
        ("0a", "w_0_2", (0, 128), (0, 51)),
        ("0b", "w_0_2", (128, 228), (0, 51)),
        ("1", "w_1_2", (0, 111), (0, 51)),
    ],
}
# target chunk -> (bias keys summed, row slice)
BIASES = {
    "0a": (("b_1_0", "b_2_0"), (0, 128)),
    "0b": (("b_1_0", "b_2_0"), (128, 228)),
    "1": (("b_0_1", "b_2_1"), (0, 111)),
    "2": (("b_0_2", "b_1_2"), (0, 51)),
}
# column offsets of each lhsT piece inside the packed weight blob [128, 942]
WOFF = {
    ("0a", "1"): 0, ("0a", "2"): 128, ("0b", "1"): 256, ("0b", "2"): 356,
    ("1", "0a"): 456, ("1", "0b"): 567, ("1", "2"): 678,
    ("2", "0a"): 789, ("2", "0b"): 840, ("2", "1"): 891,
}
WBLOB_COLS = 942
TGT_ORDER_1 = ["1", "2", "0a", "0b"]   # iter1: produce deep-chain srcs first
TGT_ORDER_2 = ["0a", "0b", "1", "2"]   # iter2: consume earliest-ready srcs
TGT_COL = {"0a": 0, "0b": 1, "1": 2, "2": 3}

LAST_RESULTS = None  # stashed BassKernelResults for test harness introspection


def _build_graph(alpha: float):
    nc = bacc.Bacc("TRN2", target_bir_lowering=False, debug=False,
                   num_devices=NCORES)
    x_ext = nc.declare_dram_parameter("x", [512, PX], BF, isOutput=False)
    y_ext = nc.declare_dram_parameter("y", [CTOT, PX], BF, isOutput=True)
    w_ext = nc.declare_dram_parameter("wblob", [128, WBLOB_COLS], BF,
                                      isOutput=False)
    b_ext = nc.declare_dram_parameter("bblob", [128, 4], F32, isOutput=False)

    with tile.TileContext(nc) as tc:
        with (
            tc.tile_pool(name="wpool", bufs=1) as wpool,
            tc.tile_pool(name="xpool", bufs=8) as xpool,
            tc.tile_pool(name="hpool", bufs=8) as hpool,
            tc.tile_pool(name="opool", bufs=8) as opool,
            tc.tile_pool(name="mpool", bufs=4) as mpool,
            tc.tile_pool(name="spool", bufs=4) as spool,
            tc.tile_pool(name="pspool", bufs=2, space="PSUM") as pspool,
        ):
            wtile = wpool.tile([128, WBLOB_COLS], BF, tag="wblob")
            nc.sync.dma_start(wtile[:], w_ext[:])
            btile = wpool.tile([128, 4], F32, tag="bblob")
            nc.sync.dma_start(btile[:], b_ext[:])
            wt = {}
            bt = {}
            for tgt, srcs in TARGETS.items():
                rows = CHUNKS[tgt][1]
                bt[tgt] = btile[0:rows, TGT_COL[tgt]:TGT_COL[tgt] + 1]
                for (src, wkey, (k0, k1), (m0, m1)) in srcs:
                    off = WOFF[(tgt, src)]
                    wt[(tgt, src)] = wtile[0:k1 - k0, off:off + (m1 - m0)]

            # PE warmup burst: ~6us of dummy matmuls while the first input
            # DMAs are in flight, so the HAM clock-gate opens before the
            # real matmul stream starts.
            wu_w = wt[("0a", "1")]
            wu_ps = pspool.tile([128, MACRO], F32, tag="ps")
            for _ in range(60):
                nc.tensor.matmul(wu_ps[0:128, 0:128], wu_w[:], wu_w[:],
                                 start=True, stop=True)

            def run_iter(src_tiles, out_pool, order):
                out = {}
                for tgt in order:
                    srcs = TARGETS[tgt]
                    rows = CHUNKS[tgt][1]
                    pt = pspool.tile([128, MACRO], F32, tag="ps")
                    # source-outer order: each weight tile stays loaded in the
                    # PE array for NSUB consecutive matmuls
                    for i, (src, wkey, (k0, k1), _m) in enumerate(srcs):
                        for n in range(NSUB):
                            c0 = n * 512
                            nc.tensor.matmul(
                                pt[0:rows, c0:c0 + 512],
                                wt[(tgt, src)],
                                src_tiles[src][0:k1 - k0, c0:c0 + 512],
                                start=(i == 0),
                                stop=(i == len(srcs) - 1),
                            )
                    msg = mpool.tile([128, MACRO], BF, tag="m")
                    nc.scalar.activation(
                        msg[0:rows, :], pt[0:rows, :],
                        mybir.ActivationFunctionType.Prelu,
                        bias=bt[tgt], scale=1.0, alpha=alpha,
                    )
                    s = spool.tile([128, MACRO], BF, tag="s")
                    nc.vector.tensor_add(s[0:rows, :], src_tiles[tgt][0:rows, :],
                                         msg[0:rows, :])
                    h = out_pool.tile([128, MACRO], BF,
                                      tag="h" if out_pool is hpool else "o")
                    nc.vector.tensor_scalar_max(h[0:rows, :], s[0:rows, :], 0.0)
                    out[tgt] = h
                return out

            for m in range(NMACRO):
                mcol = m * MACRO
                xs = {}
                for c, (g0, rows, p0) in CHUNKS.items():
                    t = xpool.tile([128, MACRO], BF, tag="x")
                    nc.sync.dma_start(t[:], x_ext[p0:p0 + 128, mcol:mcol + MACRO])
                    xs[c] = t
                h1 = run_iter(xs, hpool, TGT_ORDER_1)
                h2 = run_iter(h1, opool, TGT_ORDER_2)
                for c, (g0, rows, p0) in CHUNKS.items():
                    nc.gpsimd.dma_start(y_ext[g0:g0 + rows, mcol:mcol + MACRO],
                                        h2[c][0:rows, :])
    nc.compile()
    return nc


_GRAPH_CACHE = {}


def _get_graph(alpha: float):
    key = round(float(alpha), 8)
    if key not in _GRAPH_CACHE:
        _GRAPH_CACHE[key] = _build_graph(float(alpha))
    return _GRAPH_CACHE[key]


def _host_inputs(inputs):
    """Build per-core in_maps from full inputs."""
    xs = [np.asarray(inputs["x0"]), np.asarray(inputs["x1"]),
          np.asarray(inputs["x2"])]
    # weights / biases shared across cores, packed into single blobs
    wblob = np.zeros((128, WBLOB_COLS), dtype=BF16)
    bblob = np.zeros((128, 4), dtype=np.float32)
    for tgt, srcs in TARGETS.items():
        keys, (r0, r1) = BIASES[tgt]
        bsum = (np.asarray(inputs[keys[0]]) + np.asarray(inputs[keys[1]]))
        bblob[0:r1 - r0, TGT_COL[tgt]] = bsum[r0:r1].astype(np.float32)
        for (src, wkey, (k0, k1), (m0, m1)) in srcs:
            wT = np.asarray(inputs[wkey]).T  # [cin, cout]
            off = WOFF[(tgt, src)]
            wblob[0:k1 - k0, off:off + (m1 - m0)] = wT[k0:k1, m0:m1].astype(BF16)
    shared = {"wblob": wblob, "bblob": bblob}

    in_maps = []
    for k in range(NCORES):
        b = k // (NCORES // B)
        h0 = (k % (NCORES // B)) * ROWS_PER_CORE
        xp = np.zeros((512, PX), dtype=BF16)
        # h0 chunk a/b from xs[0], h1 from xs[1], h2 from xs[2]
        xp[0:128] = xs[0][b, 0:128, h0:h0 + ROWS_PER_CORE, :].reshape(128, PX)
        xp[128:228] = xs[0][b, 128:228, h0:h0 + ROWS_PER_CORE, :].reshape(100, PX)
        xp[256:367] = xs[1][b, :, h0:h0 + ROWS_PER_CORE, :].reshape(111, PX)
        xp[384:435] = xs[2][b, :, h0:h0 + ROWS_PER_CORE, :].reshape(51, PX)
        m = dict(shared)
        m["x"] = xp
        in_maps.append(m)
    return in_maps


def kernel(**inputs) -> np.ndarray:
    global LAST_RESULTS
    alpha = float(np.asarray(inputs["prelu_a"]).reshape(-1)[0])
    nc = _get_graph(alpha)
    in_maps = _host_inputs(inputs)
    trace = bool(os.environ.get("KERNEL_TRACE"))
    res = run_bass_kernel_spmd(nc, in_maps, list(range(NCORES)), trace=trace)
    LAST_RESULTS = res
    out = np.empty((B, CTOT, H, W), dtype=np.float32)
    for k in range(NCORES):
        b = k // (NCORES // B)
        h0 = (k % (NCORES // B)) * ROWS_PER_CORE
        y = np.asarray(res.results[k]["y"]).astype(np.float32)
        out[b, :, h0:h0 + ROWS_PER_CORE, :] = y.reshape(CTOT, ROWS_PER_CORE, W)
    return out
